# revision 1
# baseline (speedup 1.0000x reference)
"""Trainium2 Bass kernel for BatchMemoryWrapLayer (retrieval_knn).

Computation (per batch item b):
    z[n]  = cos(enc[b], mem[b,n])                 (cosine similarity)
    w     = sparsemax(z)        (shift-invariant: sparsemax(-dist) == sparsemax(z))
    mv    = sum_n w[n] * mem[b,n]
    out   = relu([enc|mv] @ W1.T + b1) @ W2.T + b2

Distribution: batch dim B=64 sharded across 8 NeuronCores (8 items/core),
MLP weights replicated. Everything runs on-device per core; no collectives.

Device strategy per core (fp16 data, fp32 accumulation):
  - mem[b] streamed HBM->SBUF in [128, 4, 1024] chunks, kept resident until
    the weighted-sum pass for that item consumed it (single HBM visit).
  - dots r[n] = mem[b,n]. xn[b]: DVE scalar_tensor_tensor fused mult+reduce
    against a partition-replicated xn.
  - sq norms: ACT activation(Square, accum_out=...).
  - sparsemax via Newton iteration on tau: f(tau) = sum relu(z - tau) - 1,
    tau' = tau + (f-1)/k.  relu-sum and support-count are single ACT
    activation(Relu/Sign, bias=-tau, accum_out=...) ops; the cross-partition
    total is one PE matmul against a ones matrix (replicates to all
    partitions).  Exactly reproduces the sort-based reference at convergence.
  - weighted sum: PE matvecs, w column stationary (M=1), mem chunks moving.
  - MLP: PE matmuls with host-pretransposed W1T/W2T streamed as moving
    operand; biases folded in as K=1 matmuls of a ones row; h transposed
    through the PE for the second matmul.
"""
import sys

for _p in ("/opt/trn_rl_repo",):
    if _p not in sys.path:
        sys.path.insert(0, _p)

import numpy as np

import concourse.bass as bass
import concourse.tile as tile
from concourse import bacc, mybir

F16 = mybir.dt.float16
F32 = mybir.dt.float32
P = 128

FULL_CFG = dict(
    n_cores=8, b_loc=8, n=4096, d=1024, d_hid=4096, d_out=1000,
    cpd=4, newton_iters=9,
)


def _segments(total, max_seg):
    segs = []
    off = 0
    while off < total:
        w = min(max_seg, total - off)
        segs.append((off, w))
        off += w
    return segs


def build_program(cfg):
    """Trace + compile the per-core program. Returns the compiled Bacc."""
    BL = cfg["b_loc"]; N = cfg["n"]; D = cfg["d"]
    DHID = cfg["d_hid"]; DOUT = cfg["d_out"]
    CPD = cfg["cpd"]; ITERS = cfg["newton_iters"]
    DIN = 2 * D
    NB = N // P                  # n-blocks of 128
    NCHUNK = NB // CPD           # DMA chunks per item
    KD = D // P                  # k-tiles per half of h_in
    KT1 = DIN // P               # k-tiles for matmul1
    KT2 = DHID // P              # k-tiles for matmul2
    GPS_SPLIT = cfg.get("gps_split", 4)   # idx%8 < split -> DVE fused, else DVE-mult+ACT-reduce
    DSEG = _segments(D, 512)
    HSEG = _segments(DHID, 512)
    OSEG = _segments(DOUT, 500)
    assert N % (P * CPD) == 0 and D % P == 0 and DHID % P == 0

    nc = bacc.Bacc("TRN2", target_bir_lowering=False, debug=False,
                   num_devices=cfg["n_cores"])

    mem_ap = nc.dram_tensor("mem", [BL, N, D], F16, kind="ExternalInput").ap()
    nrm_ap = nc.dram_tensor("nrm", [BL, P, N // P], F32, kind="ExternalInput").ap()
    xn_ap = nc.dram_tensor("xn", [BL, D], F16, kind="ExternalInput").ap()
    enct_ap = nc.dram_tensor("enct", [D, BL], F16, kind="ExternalInput").ap()
    w1t_ap = nc.dram_tensor("w1t", [DIN, DHID], F16, kind="ExternalInput").ap()
    b1_ap = nc.dram_tensor("b1r", [1, DHID], F16, kind="ExternalInput").ap()
    w2t_ap = nc.dram_tensor("w2t", [DHID, DOUT], F16, kind="ExternalInput").ap()
    b2_ap = nc.dram_tensor("b2r", [1, DOUT], F16, kind="ExternalInput").ap()
    ident_ap = nc.dram_tensor("ident", [P, P], F16, kind="ExternalInput").ap()
    out_ap = nc.dram_tensor("out", [BL, DOUT], F32, kind="ExternalOutput").ap()

    mem_v = mem_ap.rearrange("b (c p) d -> b c p d", p=P)   # [BL, NB, 128, D]

    A = mybir.AluOpType
    AF = mybir.ActivationFunctionType

    from contextlib import ExitStack
    with tile.TileContext(nc) as tc, ExitStack() as ctx:
        const_pool = ctx.enter_context(tc.tile_pool(name="const", bufs=1))
        mem_pool = ctx.enter_context(tc.tile_pool(name="memc", bufs=12 * NCHUNK // 8))
        xn_pool = ctx.enter_context(tc.tile_pool(name="xnrep", bufs=1))
        xnrow_pool = ctx.enter_context(tc.tile_pool(name="xnrow", bufs=2))
        dscr_pool = ctx.enter_context(tc.tile_pool(name="dscr", bufs=4))
        ascr_pool = ctx.enter_context(tc.tile_pool(name="ascr", bufs=4))
        nscr_pool = ctx.enter_context(tc.tile_pool(name="nscr", bufs=4))
        stat_pool = ctx.enter_context(tc.tile_pool(name="stat", bufs=3))
        small_pool = ctx.enter_context(tc.tile_pool(name="small", bufs=8))
        w1_pool = ctx.enter_context(tc.tile_pool(name="wtile", bufs=6))
        w2_pool = ctx.enter_context(tc.tile_pool(name="w2tile", bufs=6))
        mlp_pool = ctx.enter_context(tc.tile_pool(name="mlp", bufs=1))
        mvsb_pool = ctx.enter_context(tc.tile_pool(name="mvsb", bufs=2))
        mvps_pool = ctx.enter_context(tc.tile_pool(name="mvps", bufs=1, space="PSUM"))
        skps_pool = ctx.enter_context(tc.tile_pool(name="skps", bufs=1, space="PSUM"))
        mm1ps_pool = ctx.enter_context(tc.tile_pool(name="mm1ps", bufs=2, space="PSUM"))
        trps_pool = ctx.enter_context(tc.tile_pool(name="trps", bufs=1, space="PSUM"))
        mm2ps_pool = ctx.enter_context(tc.tile_pool(name="mm2ps", bufs=1, space="PSUM"))
        if True:
            # ---- constants ----
            ones_f32 = const_pool.tile([P, P], F32)
            nc.gpsimd.memset(ones_f32[:], 1.0)
            ones_row = const_pool.tile([1, BL], F16)
            nc.gpsimd.memset(ones_row[:], 1.0)
            ident_sb = const_pool.tile([BL, BL], F16)
            nc.sync.dma_start(ident_sb[:], ident_ap[0:BL, 0:BL])
            b1_sb = const_pool.tile([1, DHID], F16)
            nc.sync.dma_start(b1_sb[:], b1_ap[:])
            b2_sb = const_pool.tile([1, DOUT], F16)
            nc.sync.dma_start(b2_sb[:], b2_ap[:])
            # h_in^T tiles: [P, k, b] for enc half and mv half
            h_inT_enc = const_pool.tile([P, KD, BL], F16)
            nc.sync.dma_start(h_inT_enc[:], enct_ap.rearrange("(k p) b -> p k b", p=P))
            h_inT_mv = const_pool.tile([P, KD, BL], F16)
            # xn rows -> replicated across partitions
            xn_rep = []
            for b in range(BL):
                row = xnrow_pool.tile([1, D], F16, tag="xnrow")
                nc.sync.dma_start(row[:], xn_ap[b:b + 1, :])
                rep = xn_pool.tile([P, D], F16, tag=f"xnrep{b}")
                nc.gpsimd.partition_broadcast(rep[:], row[:])
                xn_rep.append(rep)

            # ---- per-item pipeline, software-pipelined emission ----
            # Item b's Newton/weighted-sum instructions are emitted interleaved
            # with item b+1's dot pass so each engine's in-order stream has
            # independent work between the latency-bound Newton hops.
            state = {}

            def start_item(b):
                z_b = stat_pool.tile([P, NB], F32, tag="z")
                nrm_b = stat_pool.tile([P, NB], F32, tag="nrm")
                nc.sync.dma_start(nrm_b[:], nrm_ap[b])
                neg_tau = small_pool.tile([P, 1], F32, tag="negtau")
                nc.vector.memset(neg_tau[:], 1.0 + 1.0 / N)
                state[b] = dict(z=z_b, nrm=nrm_b, nt=neg_tau, chunks=[])

            def emit_chunk(b, c):
                st = state[b]
                ch = mem_pool.tile([P, CPD, D], F16)
                nc.sync.dma_start(
                    ch[:], mem_v[b, c * CPD:(c + 1) * CPD].rearrange("c p d -> p c d"))
                st["chunks"].append(ch)
                z_b = st["z"]
                for j in range(CPD):
                    idx = c * CPD + j
                    if (idx % 8) < GPS_SPLIT:
                        # DVE: fused multiply + free-axis reduce (1x mode)
                        scr = dscr_pool.tile([P, D], F16, tag="dscr")
                        nc.vector.scalar_tensor_tensor(
                            out=scr[:], in0=ch[:, j], scalar=1.0,
                            in1=xn_rep[b][:], op0=A.mult, op1=A.mult,
                            accum_out=z_b[:, idx:idx + 1])
                    else:
                        # DVE multiply at fp16 2x, ACT reduce via accumulator
                        prod = dscr_pool.tile([P, D], F16, tag="gprod")
                        nc.vector.tensor_tensor(
                            out=prod[:], in0=ch[:, j], in1=xn_rep[b][:], op=A.mult)
                        jscr = ascr_pool.tile([P, D], F16, tag="ascr")
                        nc.scalar.activation(
                            out=jscr[:], in_=prod[:], func=AF.Copy,
                            accum_out=z_b[:, idx:idx + 1])

            def emit_newton_iter(b):
                st = state[b]
                z_b, neg_tau = st["z"], st["nt"]
                spkp = small_pool.tile([P, 2], F32, tag="spkp")
                jr = nscr_pool.tile([P, NB], F32, tag="jr")
                nc.scalar.activation(out=jr[:], in_=z_b[:], func=AF.Relu,
                                     bias=neg_tau[:, 0:1], accum_out=spkp[:, 0:1])
                js = nscr_pool.tile([P, NB], F32, tag="js")
                nc.scalar.activation(out=js[:], in_=z_b[:], func=AF.Sign,
                                     bias=neg_tau[:, 0:1], accum_out=spkp[:, 1:2])
                sk = skps_pool.tile([P, 2], F32)
                nc.tensor.matmul(sk[:], ones_f32[:], spkp[:], start=True, stop=True)
                kcol = small_pool.tile([P, 1], F32, tag="kcol")
                nc.scalar.activation(out=kcol[:], in_=sk[:, 1:2], func=AF.Copy,
                                     scale=0.5, bias=float(N) / 2.0)
                reck = small_pool.tile([P, 1], F32, tag="reck")
                nc.vector.reciprocal(reck[:], kcol[:])
                dtau = small_pool.tile([P, 1], F32, tag="dtau")
                nc.vector.scalar_tensor_tensor(
                    out=dtau[:], in0=sk[:, 0:1], scalar=-1.0, in1=reck[:],
                    op0=A.add, op1=A.mult)
                nc.vector.tensor_tensor(out=neg_tau[:], in0=neg_tau[:],
                                        in1=dtau[:], op=A.subtract)

            def emit_tail(b):
                st = state[b]
                z_b, nrm_b, neg_tau = st["z"], st["nrm"], st["nt"]
                # w' = relu(z - tau) * ||mem_n||  (folds un-normalization in)
                w_f = stat_pool.tile([P, NB], F32, tag="wf")
                nc.scalar.activation(out=w_f[:], in_=z_b[:], func=AF.Relu,
                                     bias=neg_tau[:, 0:1])
                w_b = stat_pool.tile([P, NB], F16, tag="w")
                nc.vector.tensor_tensor(out=w_b[:], in0=w_f[:], in1=nrm_b[:],
                                        op=A.mult)
                # weighted sum: mv = sum_n w'[n] yn[n, :]
                mv_ps = mvps_pool.tile([1, D], F32)
                for c in range(NCHUNK):
                    for j in range(CPD):
                        idx = c * CPD + j
                        for (s0, sw) in DSEG:
                            nc.tensor.matmul(
                                mv_ps[:, s0:s0 + sw], w_b[:, idx:idx + 1],
                                st["chunks"][c][:, j, s0:s0 + sw],
                                start=(idx == 0), stop=(idx == NB - 1))
                mv_sb = mvsb_pool.tile([1, D], F16)
                nc.scalar.copy(mv_sb[:], mv_ps[:])
                # transpose row into h_in^T column b via PE (K=1 transposes)
                for kt in range(KD):
                    trp1 = trps_pool.tile([P, 1], F16, tag="mvtr")
                    nc.tensor.transpose(trp1[:], mv_sb[:, kt * P:(kt + 1) * P],
                                        ident_sb[0:1, 0:1])
                    nc.vector.tensor_copy(h_inT_mv[:, kt, b:b + 1], trp1[:])
                del state[b]["chunks"]

            for b in range(BL):
                start_item(b)
                done = 0
                for c in range(NCHUNK):
                    emit_chunk(b, c)
                    if b > 0:
                        want = min(ITERS, (c + 1) * 2)
                        while done < want:
                            emit_newton_iter(b - 1)
                            done += 1
                if b > 0:
                    while done < ITERS:
                        emit_newton_iter(b - 1)
                        done += 1
                    emit_tail(b - 1)
            for _ in range(ITERS):
                emit_newton_iter(BL - 1)
            emit_tail(BL - 1)

            # ---- MLP ----
            h_sb = mlp_pool.tile([BL, DHID], F16)
            for hp in range(0, len(HSEG), 2):
                segs = HSEG[hp:hp + 2]
                pss = []
                for si in range(len(segs)):
                    ps1t = mm1ps_pool.tile([BL, segs[si][1]], F32, tag="ps1")
                    pss.append(ps1t)
                base = segs[0][0]
                wide = sum(hw for (_, hw) in segs)
                for k in range(KT1):
                    lhs = h_inT_enc[:, k, :] if k < KD else h_inT_mv[:, k - KD, :]
                    wt = w1_pool.tile([P, wide], F16, tag="w1t")
                    nc.sync.dma_start(wt[:], w1t_ap[k * P:(k + 1) * P, base:base + wide])
                    for si, (hs, hw) in enumerate(segs):
                        nc.tensor.matmul(pss[si][:], lhs, wt[:, hs - base:hs - base + hw],
                                         start=(k == 0), stop=False)
                for si, (hs, hw) in enumerate(segs):
                    nc.tensor.matmul(pss[si][:], ones_row[:], b1_sb[:, hs:hs + hw],
                                     start=False, stop=True)
                    nc.scalar.activation(out=h_sb[:, hs:hs + hw], in_=pss[si][:],
                                         func=AF.Relu)

            hT_sb = mlp_pool.tile([P, KT2, BL], F16)
            for kt in range(KT2):
                trp = trps_pool.tile([P, BL], F16, tag="mvtr")
                nc.tensor.transpose(trp[:], h_sb[:, kt * P:(kt + 1) * P],
                                    ident_sb[:])
                nc.vector.tensor_copy(hT_sb[:, kt, :], trp[:])

            out_sb = mlp_pool.tile([BL, DOUT], F32)
            OSEG2 = _segments(DOUT, 512)  # 512-aligned: one PSUM bank per matmul
            ps2 = mm2ps_pool.tile([BL, DOUT], F32, tag="ps2")
            for kt in range(KT2):
                wt2 = w2_pool.tile([P, DOUT], F16, tag="w2t")
                nc.sync.dma_start(wt2[:], w2t_ap[kt * P:(kt + 1) * P, :])
                for (os_, ow) in OSEG2:
                    nc.tensor.matmul(ps2[:, os_:os_ + ow], hT_sb[:, kt, :],
                                     wt2[:, os_:os_ + ow],
                                     start=(kt == 0), stop=False)
            for (os_, ow) in OSEG2:
                nc.tensor.matmul(ps2[:, os_:os_ + ow], ones_row[:],
                                 b2_sb[:, os_:os_ + ow], start=False,
                                 stop=(os_ + ow >= DOUT))
            nc.scalar.copy(out_sb[:], ps2[:])
            nc.sync.dma_start(out_ap[:], out_sb[:])

    nc.compile()
    return nc


_CACHE = {}


def _get_program(cfg_key):
    if cfg_key not in _CACHE:
        _CACHE[cfg_key] = build_program(FULL_CFG)
    return _CACHE[cfg_key]


def host_prep(encoder_output, memory_set, W1, b1, W2, b2, cfg):
    """Host-side sharding/packing. Returns (in_maps, gather_fn)."""
    n_cores = cfg["n_cores"]; BL = cfg["b_loc"]
    enc = np.asarray(encoder_output)
    B = enc.shape[0]
    assert B == n_cores * BL
    nrm = np.maximum(np.sqrt((enc.astype(np.float64) ** 2).sum(-1, keepdims=True)), 1e-6)
    xn = (enc / nrm).astype(np.float16)
    mem = np.asarray(memory_set)
    mnrm = np.sqrt(np.einsum("bnd,bnd->bn", mem, mem, optimize=True))
    mnrm = np.maximum(mnrm, 1e-6)
    mem16 = (mem / mnrm[:, :, None]).astype(np.float16)      # normalized rows
    N = mem.shape[1]
    nrm_t = np.ascontiguousarray(
        mnrm.reshape(mem.shape[0], N // 128, 128).transpose(0, 2, 1)).astype(np.float32)
    w1t = np.asarray(W1).T.astype(np.float16)          # [DIN, DHID]
    w2t = np.asarray(W2).T.astype(np.float16)          # [DHID, DOUT]
    b1r = np.asarray(b1).reshape(1, -1).astype(np.float16)
    b2r = np.asarray(b2).reshape(1, -1).astype(np.float16)
    ident = np.eye(P, dtype=np.float16)
    enct = enc.T.astype(np.float16)                    # [D, B]

    in_maps = []
    for c in range(n_cores):
        sl = slice(c * BL, (c + 1) * BL)
        in_maps.append({
            "mem": mem16[sl],
            "nrm": nrm_t[sl],
            "xn": np.ascontiguousarray(xn[sl]),
            "enct": np.ascontiguousarray(enct[:, sl]),
            "w1t": w1t, "b1r": b1r, "w2t": w2t, "b2r": b2r,
            "ident": ident,
        })
    return in_maps


def kernel(encoder_output, memory_set, W1, b1, W2, b2):
    from concourse.bass_utils import run_bass_kernel_spmd
    cfg = FULL_CFG
    nc = _get_program("full")
    in_maps = host_prep(encoder_output, memory_set, W1, b1, W2, b2, cfg)
    res = run_bass_kernel_spmd(nc, in_maps, core_ids=list(range(cfg["n_cores"])))
    out = np.concatenate([res.results[c]["out"] for c in range(cfg["n_cores"])], axis=0)
    return out.astype(np.float32)



# revision 19
# speedup vs baseline: 1.1947x; 1.1947x over previous
"""Trainium2 Bass kernel for BatchMemoryWrapLayer (retrieval_knn).

Computation (per batch item b):
    z[n]  = cos(enc[b], mem[b,n])
    w     = sparsemax(z)
    mv    = sum_n w[n] * mem[b,n]
    out   = relu([enc|mv] @ W1.T + b1) @ W2.T + b2

Distribution: batch dim B=64 sharded across 8 NeuronCores (8 items/core),
MLP weights replicated. No collectives.

Strategy (v2 — sparse-candidate): sparsemax keeps only ~90-150 of 4096 rows.
  Phase 1 (coarse): mem rows normalized, scaled, e4m3-quantized,
    host-pretransposed to [d, n] and packed for PE DoubleRow fp8 matmuls;
    z8[b] computed on the PE (M=1 matvec, K=256 per step). z8 is DMA'd
    into a wrapped [16, 256] layout (item b on partitions 16b..16b+15) so
    sparsemax-Newton runs vectorized across items: per-partition tau bias
    on ACT + one block-diag-16 PE matmul per iteration for the sums.
  Candidate select: thr = tau8 - margin; v = (z8 > thr) ? global_row : -1;
    gpsimd sparse_gather compacts candidate row indices (padded with the
    index of a shared all-zero row); indirect DMA gathers 256 raw fp16
    rows per item from HBM (one row per partition, 2 planes).
  Phase 2 (exact): per-row ssq & dot vs xn on DVE (fused mult+accum),
    z = dot * rsqrt(ssq); exact sparsemax over the 256 candidates
    (vectorized across items after a PE transpose to [16, 128]);
    w = relu(z - tau); mv = w^T @ rows (tiny PE matvecs). Zero-pad rows
    contribute exactly 0 (zero data and zero dot).
  MLP: PE matmuls with host-pretransposed fp16 weights streamed in
    column-quarters; biases folded as K=1 matmuls of a ones row.

vs v1: HBM traffic 88 -> ~60 MiB/core, and the DVE/ACT elementwise dot
pass (~270 us of engine time) is replaced by ~55 us of fp8 PE matmuls.
"""
import sys

for _p in ("/opt/trn_rl_repo",):
    if _p not in sys.path:
        sys.path.insert(0, _p)

import numpy as np

import concourse.bass as bass
import concourse.tile as tile
from concourse import bacc, mybir

F8 = mybir.dt.float8e4
F16 = mybir.dt.float16
F32 = mybir.dt.float32
I32 = mybir.dt.int32
U32 = mybir.dt.uint32
P = 128

FULL_CFG = dict(
    n_cores=8, b_loc=8, n=4096, d=1024, d_hid=4096, d_out=1000,
    scale=64.0, margin=5e-3, cand=256, p1_iters=8, p2_iters=7,
)

WQ = 16            # wrapped-z partitions per item
SENT_COLS = 16     # sentinel columns appended to v (16*16 = 256 sentinels)


def _segments(total, max_seg):
    segs = []
    off = 0
    while off < total:
        w = min(max_seg, total - off)
        segs.append((off, w))
        off += w
    return segs


def build_program(cfg):
    BL = cfg["b_loc"]; N = cfg["n"]; D = cfg["d"]
    DHID = cfg["d_hid"]; DOUT = cfg["d_out"]
    S2 = cfg["scale"] ** 2
    MARGIN = cfg["margin"]; C = cfg["cand"]
    P1_ITERS = cfg["p1_iters"]; P2_ITERS = cfg["p2_iters"]
    KD = D // P                  # 8 k-tiles per half of h_in
    KT1 = 2 * D // P             # 16 k-tiles for matmul1
    KT2 = DHID // P              # 32 k-tiles for matmul2
    NSEG = N // 512              # 8 z segments of 512
    WF = N // WQ                 # 256 wrapped free dim
    CJ = C // P                  # 2 gathered-row planes per item
    ZROW = BL * N                # shared zero-row index in mem16
    QW = 4                       # w1 column quarters
    QWID = DHID // QW            # 1024 cols per quarter
    assert C == 256 and N == 4096 and D == 1024

    nc = bacc.Bacc("TRN2", target_bir_lowering=False, debug=False,
                   num_devices=cfg["n_cores"])

    # fp8 transposed/packed normalized mem: [b][p][s][n] = yn8[b, n, 128*s+p]
    memt8_ap = nc.dram_tensor("memt8", [BL, P, KD, N], F8,
                              kind="ExternalInput").ap()
    # raw fp16 rows + one shared zero row at index BL*N
    mem16_ap = nc.dram_tensor("mem16", [BL * N + 1, D], F16,
                              kind="ExternalInput").ap()
    x8_ap = nc.dram_tensor("x8", [P, KD, 16], F8, kind="ExternalInput").ap()
    xn_ap = nc.dram_tensor("xn", [BL, D], F16, kind="ExternalInput").ap()
    enct_ap = nc.dram_tensor("enct", [D, BL], F16, kind="ExternalInput").ap()
    w1t_ap = nc.dram_tensor("w1t", [2 * D, DHID], F16, kind="ExternalInput").ap()
    b1_ap = nc.dram_tensor("b1r", [1, DHID], F16, kind="ExternalInput").ap()
    w2t_ap = nc.dram_tensor("w2t", [DHID, DOUT], F16, kind="ExternalInput").ap()
    b2_ap = nc.dram_tensor("b2r", [1, DOUT], F16, kind="ExternalInput").ap()
    ident_ap = nc.dram_tensor("ident", [P, P], F16, kind="ExternalInput").ap()
    ident32_ap = nc.dram_tensor("ident32", [P, P], F32, kind="ExternalInput").ap()
    g16_ap = nc.dram_tensor("g16", [P, P], F32, kind="ExternalInput").ap()
    g2_ap = nc.dram_tensor("g2", [2 * BL, 2 * BL], F32, kind="ExternalInput").ap()
    i16_ap = nc.dram_tensor("i16", [2 * BL, 2 * BL], F32, kind="ExternalInput").ap()
    out_ap = nc.dram_tensor("out", [BL, DOUT], F32, kind="ExternalOutput").ap()

    A = mybir.AluOpType
    AF = mybir.ActivationFunctionType

    from contextlib import ExitStack
    with tile.TileContext(nc) as tc, ExitStack() as ctx:
        const_pool = ctx.enter_context(tc.tile_pool(name="const", bufs=1))
        mem8_pool = ctx.enter_context(tc.tile_pool(name="mem8", bufs=4))
        x8_pool = ctx.enter_context(tc.tile_pool(name="x8", bufs=1))
        xn_pool = ctx.enter_context(tc.tile_pool(name="xnrep", bufs=1))
        xnrow_pool = ctx.enter_context(tc.tile_pool(name="xnrow", bufs=2))
        zw_pool = ctx.enter_context(tc.tile_pool(name="zw", bufs=1))
        nsc_pool = ctx.enter_context(tc.tile_pool(name="nsc", bufs=2))
        small_pool = ctx.enter_context(tc.tile_pool(name="small", bufs=1))
        idx_pool = ctx.enter_context(tc.tile_pool(name="idx", bufs=1))
        g_pool = ctx.enter_context(tc.tile_pool(name="grows", bufs=1))
        dsc_pool = ctx.enter_context(tc.tile_pool(name="dsc", bufs=1))
        w1_pool = ctx.enter_context(tc.tile_pool(name="w1t", bufs=22))
        w2_pool = ctx.enter_context(tc.tile_pool(name="w2t", bufs=9))
        mlp_pool = ctx.enter_context(tc.tile_pool(name="mlp", bufs=1))
        mvsb_pool = ctx.enter_context(tc.tile_pool(name="mvsb", bufs=1))
        mvrow_pool = ctx.enter_context(tc.tile_pool(name="mvrow", bufs=2))
        # PSUM pools: bank budget 2+2+2+2 = 8
        zrow_pool = ctx.enter_context(tc.tile_pool(name="zrow", bufs=2))
        zps_pool = ctx.enter_context(tc.tile_pool(name="zps", bufs=1, space="PSUM"))
        skps_pool = ctx.enter_context(tc.tile_pool(name="skps", bufs=1, space="PSUM"))
        trp_pool = ctx.enter_context(tc.tile_pool(name="trp", bufs=1, space="PSUM"))
        trmv_pool = ctx.enter_context(tc.tile_pool(name="trmv", bufs=1, space="PSUM"))
        bigps_pool = ctx.enter_context(tc.tile_pool(name="bigps", bufs=1, space="PSUM"))
        mm1ps_pool = ctx.enter_context(tc.tile_pool(name="mm1ps", bufs=1, space="PSUM"))

        # ---------------- constants ----------------
        ident_sb = const_pool.tile([P, P], F16)
        nc.sync.dma_start(ident_sb[:], ident_ap[:])
        ident32_sb = const_pool.tile([P, P], F32)
        nc.sync.dma_start(ident32_sb[:], ident32_ap[:])
        g16_sb = const_pool.tile([P, P], F32)
        nc.sync.dma_start(g16_sb[:], g16_ap[:])
        g2_sb = const_pool.tile([2 * BL, 2 * BL], F32)
        nc.sync.dma_start(g2_sb[:], g2_ap[:])
        i16_sb = const_pool.tile([2 * BL, 2 * BL], F32)
        nc.sync.dma_start(i16_sb[:], i16_ap[:])
        ones_row = const_pool.tile([1, BL], F16)
        nc.gpsimd.memset(ones_row[:], 1.0)
        ones16_128 = const_pool.tile([2 * BL, P], F32)
        nc.gpsimd.memset(ones16_128[:], 1.0)
        b1_sb = const_pool.tile([1, DHID], F16)
        nc.sync.dma_start(b1_sb[:], b1_ap[:])
        b2_sb = const_pool.tile([1, DOUT], F16)
        nc.sync.dma_start(b2_sb[:], b2_ap[:])
        h_inT_enc = const_pool.tile([P, KD, BL], F16)
        nc.sync.dma_start(h_inT_enc[:], enct_ap.rearrange("(k p) b -> p k b", p=P))
        h_inT_mv = const_pool.tile([P, KD, BL], F16)
        # candidate index map: value at (p, f) = p*WF + f + 1  (global row + 1)
        idxp1_i = const_pool.tile([P, WF], I32)
        nc.gpsimd.iota(idxp1_i[:], pattern=[[1, WF]], base=1, channel_multiplier=WF)
        idxp1 = const_pool.tile([P, WF], F32)
        nc.vector.tensor_copy(idxp1[:], idxp1_i[:])

        # candidate-select workspace (all items): [128, WF + SENT_COLS]
        v_all = zw_pool.tile([P, WF + SENT_COLS], F32, tag="vall")
        nc.vector.memset(v_all[:, WF:], float(ZROW))
        zw = zw_pool.tile([P, WF], F32, tag="zw")
        # phase-1 newton state (per-partition, item = p//16)
        negtau = small_pool.tile([P, 1], F32, tag="negtau")
        nc.vector.memset(negtau[:], 1.0 + 1.0 / N)
        spk = small_pool.tile([P, 2], F32, tag="spk")
        nc.vector.memset(spk[:], 0.0)
        kcol = small_pool.tile([P, 1], F32, tag="kcol")
        reck = small_pool.tile([P, 1], F32, tag="reck")
        dtau = small_pool.tile([P, 1], F32, tag="dtau")
        thrs = small_pool.tile([P, 1], F32, tag="thrs")
        # phase-2 state
        negtau2 = small_pool.tile([2 * BL, 1], F32, tag="negtau2")
        nc.vector.memset(negtau2[:], 1.0 + 1.0 / C)
        spk2 = small_pool.tile([2 * BL, 2], F32, tag="spk2")
        kcol2 = small_pool.tile([2 * BL, 1], F32, tag="kcol2")
        reck2 = small_pool.tile([2 * BL, 1], F32, tag="reck2")
        dtau2 = small_pool.tile([2 * BL, 1], F32, tag="dtau2")
        diag16 = small_pool.tile([2 * BL, 2 * BL], F32, tag="diag16")
        ntrep = small_pool.tile([P, 2 * BL], F32, tag="ntrep")
        zcT = small_pool.tile([2 * BL, P], F32, tag="zcT")
        ssq_all = small_pool.tile([P, 2 * BL], F32, tag="ssq")
        dot_all = small_pool.tile([P, 2 * BL], F32, tag="dot")
        rinv_all = small_pool.tile([P, 2 * BL], F32, tag="rinv")
        zc_all = small_pool.tile([P, 2 * BL], F32, tag="zc")
        w_all = small_pool.tile([P, 2 * BL], F16, tag="wall")
        eps_col = small_pool.tile([P, 1], F32, tag="eps")
        nc.vector.memset(eps_col[:], 1e-12)

        # per-item persistent tiles
        x8_sb = x8_pool.tile([P, KD, 16], F8, tag="x8")
        nc.sync.dma_start(x8_sb[:], x8_ap[:])
        xn_rep = []
        for b in range(BL):
            row = xnrow_pool.tile([1, D], F16, tag="xnrow")
            nc.sync.dma_start(row[:], xn_ap[b:b + 1, :])
            rep = xn_pool.tile([P, D], F16, tag=f"xnrep{b}")
            nc.gpsimd.partition_broadcast(rep[:], row[:])
            xn_rep.append(rep)

        G_tiles = {}
        idx32 = idx_pool.tile([P, BL, CJ], I32, tag="idx32")
        nf_all = idx_pool.tile([1, 1], U32, tag="nf")
        mv_cat = mvsb_pool.tile([BL, D], F16, tag="mvcat")

        # ---------------- phase A: fp8 dots on PE ----------------
        def emit_dots(b):
            chunks = []
            for t in range(KD // 2):
                ch = mem8_pool.tile([P, 2, N], F8)
                nc.sync.dma_start(ch[:], memt8_ap[b, :, 2 * t:2 * t + 2, :])
                chunks.append(ch)
            for s in range(NSEG):
                zps = zps_pool.tile([1, 512], F32)
                for t in range(KD // 2):
                    nc.tensor.matmul(
                        zps[:], x8_sb[:, 2 * t:2 * t + 2, b:b + 1],
                        chunks[t][:, :, 512 * s:512 * (s + 1)],
                        start=(t == 0), stop=(t == KD // 2 - 1),
                        perf_mode=mybir.MatmulPerfMode.DoubleRow)
                # wrapped copy: zw[16b + n//256, n%256] = z[n]
                zrow = zrow_pool.tile([1, 512], F32, tag="zrow")
                nc.scalar.copy(zrow[:], zps[:])
                for o in range(2):
                    q = WQ * b + 2 * s + o
                    nc.sync.dma_start(zw[q:q + 1, :],
                                      zrow[0:1, 256 * o:256 * o + 256])

        # ---------------- phase B: coarse newton (one iteration) ----------
        def emit_newton1_iter(g):
            lo, hi = 64 * g, 64 * g + 64
            jr = nsc_pool.tile([P, WF], F32, tag=f"jr{g}")
            nc.scalar.activation(out=jr[lo:hi, :], in_=zw[lo:hi, :], func=AF.Relu,
                                 scale=1.0 / S2, bias=negtau[lo:hi, 0:1],
                                 accum_out=spk[lo:hi, 0:1])
            js = nsc_pool.tile([P, WF], F32, tag=f"js{g}")
            nc.scalar.activation(out=js[lo:hi, :], in_=zw[lo:hi, :], func=AF.Sign,
                                 scale=1.0 / S2, bias=negtau[lo:hi, 0:1],
                                 accum_out=spk[lo:hi, 1:2])
            skp = skps_pool.tile([P, 16], F32, tag="skps")
            nc.tensor.matmul(skp[:, 0:2], g16_sb[:], spk[:], start=True, stop=True)
            nc.scalar.activation(out=kcol[lo:hi], in_=skp[lo:hi, 1:2],
                                 func=AF.Copy, scale=0.5, bias=float(N) / 2.0)
            nc.vector.reciprocal(reck[lo:hi], kcol[lo:hi])
            nc.vector.scalar_tensor_tensor(
                out=dtau[lo:hi], in0=skp[lo:hi, 0:1], scalar=-1.0,
                in1=reck[lo:hi], op0=A.add, op1=A.mult)
            nc.vector.tensor_tensor(out=negtau[lo:hi], in0=negtau[lo:hi],
                                    in1=dtau[lo:hi], op=A.subtract)

        # ---------------- phase C: candidate select/compact/gather --------
        def emit_candidates(g):
            lo, hi = 64 * g, 64 * g + 64
            # thr_scaled = (tau - margin) * S2 = (negtau + margin) * (-S2)
            nc.vector.tensor_scalar(
                out=thrs[lo:hi], in0=negtau[lo:hi], scalar1=MARGIN,
                scalar2=-S2, op0=A.add, op1=A.mult)
            mask = nsc_pool.tile([P, WF], F32, tag=f"mask{g}")
            nc.vector.tensor_scalar(
                out=mask[lo:hi, :], in0=zw[lo:hi, :], scalar1=thrs[lo:hi, 0:1],
                scalar2=None, op0=A.is_gt)
            nc.vector.tensor_tensor(out=v_all[lo:hi, :WF], in0=mask[lo:hi, :],
                                    in1=idxp1[lo:hi, :], op=A.mult)
            nc.vector.tensor_scalar(
                out=v_all[lo:hi, :WF], in0=v_all[lo:hi, :WF], scalar1=1.0,
                scalar2=None, op0=A.subtract)
            for b in range(4 * g, 4 * g + 4):
                qb = WQ * b
                vstage = idx_pool.tile([WQ, WF + SENT_COLS], F32,
                                       tag=f"vstage{b % 2}")
                nc.sync.dma_start(vstage[:], v_all[qb:qb + WQ, :])
                idxw = idx_pool.tile([WQ, C // WQ], F32, tag=f"idxw{b % 2}")
                nc.gpsimd.sparse_gather(
                    idxw[:], vstage[:], num_found=nf_all[0:1, 0:1])
                idxi = idx_pool.tile([WQ, C // WQ], I32, tag=f"idxi{b % 2}")
                nc.vector.tensor_copy(idxi[:], idxw[:])
                # spread 256 indices one-per-partition: [16,16] -> [128,2]
                for r in range(8):
                    nc.sync.dma_start(idx32[WQ * r:WQ * r + WQ, b, :],
                                      idxi[:, 2 * r:2 * r + 2])
                G = g_pool.tile([P, CJ, D], F16, tag=f"G{b}")
                G_tiles[b] = G
                for j in range(CJ):
                    nc.gpsimd.indirect_dma_start(
                        out=G[:, j], out_offset=None,
                        in_=mem16_ap,
                        in_offset=bass.IndirectOffsetOnAxis(
                            ap=idx32[:, b, j:j + 1], axis=0),
                    )

        # ---------------- phase D1: exact per-row stats -------------------
        def emit_exact_stats(b):
            G = G_tiles[b]
            for j in range(CJ):
                scr = dsc_pool.tile([P, D], F16, tag="dsc")
                nc.vector.scalar_tensor_tensor(
                    out=scr[:], in0=G[:, j], scalar=1.0, in1=G[:, j],
                    op0=A.mult, op1=A.mult,
                    accum_out=ssq_all[:, 2 * b + j:2 * b + j + 1])
                scr2 = dsc_pool.tile([P, D], F16, tag="dsc2")
                nc.vector.scalar_tensor_tensor(
                    out=scr2[:], in0=G[:, j], scalar=1.0, in1=xn_rep[b][:],
                    op0=A.mult, op1=A.mult,
                    accum_out=dot_all[:, 2 * b + j:2 * b + j + 1])

        # ---------------- phase D2: exact sparsemax (all items) -----------
        def emit_exact_sparsemax():
            nc.scalar.activation(out=rinv_all[:], in_=ssq_all[:], func=AF.Sqrt,
                                 bias=eps_col[:, 0:1])
            nc.vector.reciprocal(rinv_all[:], rinv_all[:])
            nc.vector.tensor_tensor(out=zc_all[:], in0=dot_all[:],
                                    in1=rinv_all[:], op=A.mult)
            trp = trp_pool.tile([2 * BL, P], F32, tag="trp")
            nc.tensor.transpose(trp[:], zc_all[:], ident32_sb[:])
            nc.vector.tensor_copy(zcT[:], trp[:])
            for _ in range(P2_ITERS):
                jr = nsc_pool.tile([2 * BL, P], F32, tag="jr2")
                nc.scalar.activation(out=jr[:], in_=zcT[:], func=AF.Relu,
                                     bias=negtau2[:, 0:1],
                                     accum_out=spk2[:, 0:1])
                js = nsc_pool.tile([2 * BL, P], F32, tag="js2")
                nc.scalar.activation(out=js[:], in_=zcT[:], func=AF.Sign,
                                     bias=negtau2[:, 0:1],
                                     accum_out=spk2[:, 1:2])
                skpt = skps_pool.tile([P, 16], F32, tag="skps")
                skp = skpt[0:2 * BL, 0:2]
                nc.tensor.matmul(skp, g2_sb[:], spk2[:], start=True, stop=True)
                nc.scalar.activation(out=kcol2[:], in_=skp[:, 1:2],
                                     func=AF.Copy, scale=0.5, bias=float(C) / 2.0)
                nc.vector.reciprocal(reck2[:], kcol2[:])
                nc.vector.scalar_tensor_tensor(
                    out=dtau2[:], in0=skp[:, 0:1], scalar=-1.0,
                    in1=reck2[:], op0=A.add, op1=A.mult)
                nc.vector.tensor_tensor(out=negtau2[:], in0=negtau2[:],
                                        in1=dtau2[:], op=A.subtract)
            # replicate -tau_b to all partitions: ntrep[:, c] = negtau2[c]
            nc.vector.tensor_scalar(out=diag16[:], in0=i16_sb[:],
                                    scalar1=negtau2[:, 0:1], scalar2=None,
                                    op0=A.mult)
            ntpst = skps_pool.tile([P, 16], F32, tag="skps")
            nc.tensor.matmul(ntpst[:, 0:2 * BL], ones16_128[:], diag16[:],
                             start=True, stop=True)
            nc.vector.tensor_copy(ntrep[:], ntpst[:, 0:2 * BL])
            # weights: w = relu(zc - tau)  (fp16)
            for b in range(BL):
                nc.scalar.activation(out=w_all[:, 2 * b:2 * b + 2],
                                     in_=zc_all[:, 2 * b:2 * b + 2],
                                     func=AF.Relu, bias=ntrep[:, 2 * b:2 * b + 1])

        # ---------------- phase E: weighted sum + h_inT ------------------
        def emit_wsum(b):
            G = G_tiles[b]
            mv_ps = bigps_pool.tile([BL, D], F32, tag="mvps")
            for (s0, sw) in _segments(D, 512):
                for j in range(CJ):
                    nc.tensor.matmul(
                        mv_ps[0:1, s0:s0 + sw], w_all[:, 2 * b + j:2 * b + j + 1],
                        G[:, j, s0:s0 + sw], start=(j == 0), stop=(j == CJ - 1))
            mvrow = mvrow_pool.tile([1, D], F16, tag="mvrow")
            nc.scalar.copy(mvrow[:], mv_ps[0:1, :])
            nc.sync.dma_start(mv_cat[b:b + 1, :], mvrow[:])

        def emit_hT_mv():
            for kt in range(KD):
                trp = trmv_pool.tile([P, BL], F16, tag="trmv")
                nc.tensor.transpose(trp[:], mv_cat[:, kt * P:(kt + 1) * P],
                                    ident_sb[0:BL, 0:BL])
                nc.vector.tensor_copy(h_inT_mv[:, kt, :], trp[:])

        # ---------------- emission schedule ----------------
        for b in range(4):
            emit_dots(b)
        g0_done = 0
        for b in range(4, BL):
            emit_dots(b)
            # interleave group-0 newton with items 4..6 dots; candidates
            # after item 6 so gathers overlap item-7 streaming
            if b < 6:
                for _ in range(4):
                    emit_newton1_iter(0)
                    g0_done += 1
            elif b == 6:
                while g0_done < P1_ITERS:
                    emit_newton1_iter(0)
                    g0_done += 1
                emit_candidates(0)

        # W streams start right after the last mem8 chunk DMA
        w1_tiles = {}
        for q in range(QW):
            for k in range(KT1):
                wt = w1_pool.tile([P, QWID], F16, tag="w1t")
                nc.sync.dma_start(
                    wt[:], w1t_ap[k * P:(k + 1) * P, q * QWID:(q + 1) * QWID])
                w1_tiles[(q, k)] = wt
        w2_tiles = {}
        for kt in range(KT2):
            wt2 = w2_pool.tile([P, DOUT], F16, tag="w2t")
            nc.sync.dma_start(wt2[:], w2t_ap[kt * P:(kt + 1) * P, :])
            w2_tiles[kt] = wt2

        for _ in range(P1_ITERS):
            emit_newton1_iter(1)
        emit_candidates(1)
        for bb in range(BL):
            emit_exact_stats(bb)
        emit_exact_sparsemax()
        for b in range(BL):
            emit_wsum(b)
        emit_hT_mv()

        # ---------------- MLP ----------------
        h_sb = mlp_pool.tile([BL, DHID], F16)
        for q in range(QW):
            segs = _segments(QWID, 512)
            ps1 = mm1ps_pool.tile([BL, QWID], F32, tag="ps1")
            for k in range(KT1):
                lhs = h_inT_enc[:, k, :] if k < KD else h_inT_mv[:, k - KD, :]
                wt = w1_tiles[(q, k)]
                for (hs, hw) in segs:
                    nc.tensor.matmul(ps1[:, hs:hs + hw], lhs, wt[:, hs:hs + hw],
                                     start=(k == 0), stop=False)
            for (hs, hw) in segs:
                h0 = q * QWID + hs
                nc.tensor.matmul(ps1[:, hs:hs + hw], ones_row[:],
                                 b1_sb[:, h0:h0 + hw], start=False, stop=True)
                nc.scalar.activation(out=h_sb[:, h0:h0 + hw], in_=ps1[:, hs:hs + hw],
                                     func=AF.Relu)

        hT_sb = mlp_pool.tile([P, KT2, BL], F16)
        for kt in range(KT2):
            trp = trmv_pool.tile([P, BL], F16, tag="trmv")
            nc.tensor.transpose(trp[:], h_sb[:, kt * P:(kt + 1) * P],
                                ident_sb[0:BL, 0:BL])
            nc.vector.tensor_copy(hT_sb[:, kt, :], trp[:])

        out_sb = mlp_pool.tile([BL, DOUT], F32)
        OSEG2 = _segments(DOUT, 512)
        ps2 = bigps_pool.tile([BL, D], F32, tag="mvps")
        for kt in range(KT2):
            wt2 = w2_tiles[kt]
            for (os_, ow) in OSEG2:
                nc.tensor.matmul(ps2[:, os_:os_ + ow], hT_sb[:, kt, :],
                                 wt2[:, os_:os_ + ow],
                                 start=(kt == 0), stop=False)
        for (os_, ow) in OSEG2:
            nc.tensor.matmul(ps2[:, os_:os_ + ow], ones_row[:],
                             b2_sb[:, os_:os_ + ow], start=False,
                             stop=(os_ + ow >= DOUT))
        nc.scalar.copy(out_sb[:], ps2[:, :DOUT])
        nc.sync.dma_start(out_ap[:], out_sb[:])

    nc.compile()
    return nc


_CACHE = {}


def _get_program(cfg_key):
    if cfg_key not in _CACHE:
        _CACHE[cfg_key] = build_program(FULL_CFG)
    return _CACHE[cfg_key]


def host_prep(encoder_output, memory_set, W1, b1, W2, b2, cfg):
    from ml_dtypes import float8_e4m3fn
    n_cores = cfg["n_cores"]; BL = cfg["b_loc"]; S = cfg["scale"]
    enc = np.asarray(encoder_output, dtype=np.float32)
    mem = np.asarray(memory_set, dtype=np.float32)
    B, N, D = mem.shape
    assert B == n_cores * BL
    nrm = np.maximum(np.sqrt((enc ** 2).sum(-1, keepdims=True)), 1e-6)
    xn = enc / nrm
    mnrm = np.maximum(np.sqrt(np.einsum("bnd,bnd->bn", mem, mem)), 1e-6)
    yn = mem / mnrm[:, :, None]

    def q8(x):
        return np.clip(x * S, -240, 240).astype(float8_e4m3fn)

    # memt8[b, p, s, n] = q8(yn[b, n, 128 s + p])
    yn8 = q8(yn).reshape(B, N, D // P, P)             # [b, n, s, p]
    memt8 = np.ascontiguousarray(yn8.transpose(0, 3, 2, 1))
    x8q = q8(xn).reshape(B, D // P, P)                # [b, s, p]
    x8 = np.zeros((n_cores, P, D // P, 16), dtype=float8_e4m3fn)
    for c in range(n_cores):
        x8[c, :, :, :BL] = x8q[c * BL:(c + 1) * BL].transpose(2, 1, 0)
    mem16 = mem.astype(np.float16).reshape(n_cores, BL * N, D)
    zrow = np.zeros((n_cores, 1, D), np.float16)
    mem16 = np.concatenate([mem16, zrow], axis=1)     # [cores, BL*N+1, D]
    xn16 = xn.astype(np.float16)
    enct = enc.T.astype(np.float16)
    w1t = np.asarray(W1).T.astype(np.float16)
    w2t = np.asarray(W2).T.astype(np.float16)
    b1r = np.asarray(b1).reshape(1, -1).astype(np.float16)
    b2r = np.asarray(b2).reshape(1, -1).astype(np.float16)
    ident = np.eye(P, dtype=np.float16)
    ident32 = np.eye(P, dtype=np.float32)
    g16 = np.kron(np.eye(P // 16, dtype=np.float32),
                  np.ones((16, 16), np.float32))
    g2 = np.kron(np.eye(BL, dtype=np.float32), np.ones((2, 2), np.float32))
    i16 = np.eye(2 * BL, dtype=np.float32)

    in_maps = []
    for c in range(n_cores):
        sl = slice(c * BL, (c + 1) * BL)
        in_maps.append({
            "memt8": memt8[sl],
            "mem16": mem16[c],
            "x8": x8[c],
            "xn": np.ascontiguousarray(xn16[sl]),
            "enct": np.ascontiguousarray(enct[:, sl]),
            "w1t": w1t, "b1r": b1r, "w2t": w2t, "b2r": b2r,
            "ident": ident, "ident32": ident32,
            "g16": g16, "g2": g2, "i16": i16,
        })
    return in_maps


def kernel(encoder_output, memory_set, W1, b1, W2, b2):
    from concourse.bass_utils import run_bass_kernel_spmd
    cfg = FULL_CFG
    nc = _get_program("full")
    in_maps = host_prep(encoder_output, memory_set, W1, b1, W2, b2, cfg)
    res = run_bass_kernel_spmd(nc, in_maps, core_ids=list(range(cfg["n_cores"])))
    out = np.concatenate([res.results[c]["out"] for c in range(cfg["n_cores"])], axis=0)
    return out.astype(np.float32)


# revision 22
# speedup vs baseline: 1.4866x; 1.2443x over previous
"""Trainium2 Bass kernel for BatchMemoryWrapLayer (retrieval_knn).

Computation (per batch item b):
    z[n]  = cos(enc[b], mem[b,n])
    w     = sparsemax(z)
    mv    = sum_n w[n] * mem[b,n]
    out   = relu([enc|mv] @ W1.T + b1) @ W2.T + b2

Distribution: batch dim B=64 sharded across 8 NeuronCores (8 items/core),
MLP weights replicated. No collectives.

Strategy (v3 — sparse-candidate): sparsemax keeps only ~90-150 of 4096 rows.
  Phase 1 (coarse): mem rows normalized, scaled, e4m3-quantized,
    host-pretransposed to [d, n] and packed for PE DoubleRow fp8 matmuls;
    z8[b] computed on the PE (M=1 matvec, K=256 per step). z8 is staged to
    SBUF and DMA'd into a wrapped [16, 256] layout (item b on partitions
    16b..16b+15) so sparsemax-Newton runs vectorized across items
    (per-partition tau bias on ACT + one block-diag-16 PE matmul/iter).
  Candidate select: thr = tau8 - margin; v = (z8 > thr) ? global_row : -1;
    gpsimd sparse_gather compacts candidate row indices (padded with the
    index of a shared all-zero row); indirect DMA gathers 256 raw fp16
    rows per item from HBM (one row per partition, 2 planes). Each row
    carries its precomputed 1/norm at column 1024 (pad row: 0).
  Phase 2 (exact): per-row dot vs xn on DVE (fused mult+accum),
    z = dot * rinv; exact sparsemax over the 256 candidates (vectorized
    across items after one PE transpose to [16, 128]); w = relu(z - tau);
    mv = w^T @ rows (tiny PE matvecs). Zero-pad rows contribute 0.
  MLP: split matmul1 — the enc half of h runs early (overlapping the
    retrieval tail) into h_enc; the mv half runs last and is fused with
    h_enc + relu on DVE/ACT. Weights fp16, streamed; biases folded as
    K=1 matmuls of a ones row.
"""
import sys

for _p in ("/opt/trn_rl_repo",):
    if _p not in sys.path:
        sys.path.insert(0, _p)

import numpy as np

import concourse.bass as bass
import concourse.tile as tile
from concourse import bacc, mybir

F8 = mybir.dt.float8e4
F16 = mybir.dt.float16
F32 = mybir.dt.float32
I32 = mybir.dt.int32
U32 = mybir.dt.uint32
P = 128

FULL_CFG = dict(
    n_cores=8, b_loc=8, n=4096, d=1024, d_hid=4096, d_out=1000,
    scale=64.0, margin=7e-3, cand=256, p1_iters=6, p2_iters=6,
)

WQ = 16            # wrapped-z partitions per item
SENT_COLS = 16     # sentinel columns appended to v (16*16 = 256 sentinels)
DW = 1032          # gathered row width: 1024 data + rinv @1024 + pad


def _segments(total, max_seg):
    segs = []
    off = 0
    while off < total:
        w = min(max_seg, total - off)
        segs.append((off, w))
        off += w
    return segs


def build_program(cfg):
    BL = cfg["b_loc"]; N = cfg["n"]; D = cfg["d"]
    DHID = cfg["d_hid"]; DOUT = cfg["d_out"]
    S2 = cfg["scale"] ** 2
    MARGIN = cfg["margin"]; C = cfg["cand"]
    P1_ITERS = cfg["p1_iters"]; P2_ITERS = cfg["p2_iters"]
    KD = D // P                  # 8 k-tiles per half of h_in
    KT1 = 2 * D // P             # 16 k-tiles for matmul1
    KT2 = DHID // P              # 32 k-tiles for matmul2
    NSEG = N // 512              # 8 z segments of 512
    WF = N // WQ                 # 256 wrapped free dim
    CJ = C // P                  # 2 gathered-row planes per item
    ZROW = BL * N                # shared zero-row index in mem16
    QW = 4                       # w1 column quarters
    QWID = DHID // QW            # 1024 cols per quarter
    assert C == 256 and N == 4096 and D == 1024

    nc = bacc.Bacc("TRN2", target_bir_lowering=False, debug=False,
                   num_devices=cfg["n_cores"])

    # fp8 transposed/packed normalized mem: [b][p][s][n] = yn8[b, n, 128*s+p]
    memt8_ap = nc.dram_tensor("memt8", [BL, P, KD, N], F8,
                              kind="ExternalInput").ap()
    # raw fp16 rows (+ rinv at col 1024) + shared zero row at index BL*N
    mem16_ap = nc.dram_tensor("mem16", [BL * N + 1, DW], F16,
                              kind="ExternalInput").ap()
    x8_ap = nc.dram_tensor("x8", [P, KD, 16], F8, kind="ExternalInput").ap()
    xn_ap = nc.dram_tensor("xn", [BL, D], F16, kind="ExternalInput").ap()
    enct_ap = nc.dram_tensor("enct", [D, BL], F16, kind="ExternalInput").ap()
    w1t_ap = nc.dram_tensor("w1t", [2 * D, DHID], F16, kind="ExternalInput").ap()
    b1_ap = nc.dram_tensor("b1r", [1, DHID], F16, kind="ExternalInput").ap()
    w2t_ap = nc.dram_tensor("w2t", [DHID, DOUT], F16, kind="ExternalInput").ap()
    b2_ap = nc.dram_tensor("b2r", [1, DOUT], F16, kind="ExternalInput").ap()
    ident_ap = nc.dram_tensor("ident", [P, P], F16, kind="ExternalInput").ap()
    ident32_ap = nc.dram_tensor("ident32", [P, P], F32, kind="ExternalInput").ap()
    g16_ap = nc.dram_tensor("g16", [P, P], F32, kind="ExternalInput").ap()
    g2_ap = nc.dram_tensor("g2", [2 * BL, 2 * BL], F32, kind="ExternalInput").ap()
    i16_ap = nc.dram_tensor("i16", [2 * BL, 2 * BL], F32, kind="ExternalInput").ap()
    out_ap = nc.dram_tensor("out", [BL, DOUT], F32, kind="ExternalOutput").ap()

    A = mybir.AluOpType
    AF = mybir.ActivationFunctionType

    from contextlib import ExitStack
    with tile.TileContext(nc) as tc, ExitStack() as ctx:
        const_pool = ctx.enter_context(tc.tile_pool(name="const", bufs=1))
        mem8_pool = ctx.enter_context(tc.tile_pool(name="mem8", bufs=10))
        x8_pool = ctx.enter_context(tc.tile_pool(name="x8", bufs=1))
        xn_pool = ctx.enter_context(tc.tile_pool(name="xnrep", bufs=1))
        xnrow_pool = ctx.enter_context(tc.tile_pool(name="xnrow", bufs=2))
        zw_pool = ctx.enter_context(tc.tile_pool(name="zw", bufs=1))
        zrow_pool = ctx.enter_context(tc.tile_pool(name="zrow", bufs=2))
        nsc_pool = ctx.enter_context(tc.tile_pool(name="nsc", bufs=2))
        small_pool = ctx.enter_context(tc.tile_pool(name="small", bufs=1))
        idx_pool = ctx.enter_context(tc.tile_pool(name="idx", bufs=1))
        g_pool = ctx.enter_context(tc.tile_pool(name="grows", bufs=1))
        dsc_pool = ctx.enter_context(tc.tile_pool(name="dsc", bufs=2))
        w1_pool = ctx.enter_context(tc.tile_pool(name="w1t", bufs=12))
        w2_pool = ctx.enter_context(tc.tile_pool(name="w2t", bufs=8))
        mlp_pool = ctx.enter_context(tc.tile_pool(name="mlp", bufs=1))
        mvsb_pool = ctx.enter_context(tc.tile_pool(name="mvsb", bufs=1))
        mvrow_pool = ctx.enter_context(tc.tile_pool(name="mvrow", bufs=2))
        # PSUM pools: bank budget 2+1+1+2+2 = 8
        zps_pool = ctx.enter_context(tc.tile_pool(name="zps", bufs=2, space="PSUM"))
        sps_pool = ctx.enter_context(tc.tile_pool(name="sps", bufs=1, space="PSUM"))
        trmv_pool = ctx.enter_context(tc.tile_pool(name="trmv", bufs=1, space="PSUM"))
        bigps_pool = ctx.enter_context(tc.tile_pool(name="bigps", bufs=1, space="PSUM"))
        mm1ps_pool = ctx.enter_context(tc.tile_pool(name="mm1ps", bufs=1, space="PSUM"))

        # ---------------- constants ----------------
        ident_sb = const_pool.tile([P, P], F16)
        nc.sync.dma_start(ident_sb[:], ident_ap[:])
        ident32_sb = const_pool.tile([P, P], F32)
        nc.sync.dma_start(ident32_sb[:], ident32_ap[:])
        g16_sb = const_pool.tile([P, P], F32)
        nc.sync.dma_start(g16_sb[:], g16_ap[:])
        g2_sb = const_pool.tile([2 * BL, 2 * BL], F32)
        nc.sync.dma_start(g2_sb[:], g2_ap[:])
        i16_sb = const_pool.tile([2 * BL, 2 * BL], F32)
        nc.sync.dma_start(i16_sb[:], i16_ap[:])
        ones_row = const_pool.tile([1, BL], F16)
        nc.gpsimd.memset(ones_row[:], 1.0)
        ones16_128 = const_pool.tile([2 * BL, P], F32)
        nc.gpsimd.memset(ones16_128[:], 1.0)
        b1_sb = const_pool.tile([1, DHID], F16)
        nc.sync.dma_start(b1_sb[:], b1_ap[:])
        b2_sb = const_pool.tile([1, DOUT], F16)
        nc.sync.dma_start(b2_sb[:], b2_ap[:])
        h_inT_enc = const_pool.tile([P, KD, BL], F16)
        nc.sync.dma_start(h_inT_enc[:], enct_ap.rearrange("(k p) b -> p k b", p=P))
        h_inT_mv = const_pool.tile([P, KD, BL], F16)
        # candidate index map: value at (p, f) = p*WF + f + 1  (global row + 1)
        idxp1_i = const_pool.tile([P, WF], I32)
        nc.gpsimd.iota(idxp1_i[:], pattern=[[1, WF]], base=1, channel_multiplier=WF)
        idxp1 = const_pool.tile([P, WF], F32)
        nc.vector.tensor_copy(idxp1[:], idxp1_i[:])

        # candidate-select workspace (all items): [128, WF + SENT_COLS]
        v_all = zw_pool.tile([P, WF + SENT_COLS], F32, tag="vall")
        nc.vector.memset(v_all[:, WF:], float(ZROW))
        zw = zw_pool.tile([P, WF], F32, tag="zw")
        # phase-1 newton state (per-partition, item = p//16)
        negtau = small_pool.tile([P, 1], F32, tag="negtau")
        nc.vector.memset(negtau[:], 1.0 + 1.0 / N)
        spk = small_pool.tile([P, 2], F32, tag="spk")
        nc.vector.memset(spk[:], 0.0)
        kcol = small_pool.tile([P, 1], F32, tag="kcol")
        reck = small_pool.tile([P, 1], F32, tag="reck")
        dtau = small_pool.tile([P, 1], F32, tag="dtau")
        thrs = small_pool.tile([P, 1], F32, tag="thrs")
        # phase-2 state
        negtau2 = small_pool.tile([2 * BL, 1], F32, tag="negtau2")
        nc.vector.memset(negtau2[:], 1.0 + 1.0 / C)
        spk2 = small_pool.tile([2 * BL, 2], F32, tag="spk2")
        kcol2 = small_pool.tile([2 * BL, 1], F32, tag="kcol2")
        reck2 = small_pool.tile([2 * BL, 1], F32, tag="reck2")
        dtau2 = small_pool.tile([2 * BL, 1], F32, tag="dtau2")
        diag16 = small_pool.tile([2 * BL, 2 * BL], F32, tag="diag16")
        ntrep = small_pool.tile([P, 2 * BL], F32, tag="ntrep")
        zcT = small_pool.tile([2 * BL, P], F32, tag="zcT")
        dot_all = small_pool.tile([P, 2 * BL], F32, tag="dot")
        rinv_all = small_pool.tile([P, 2 * BL], F32, tag="rinv")
        zc_all = small_pool.tile([P, 2 * BL], F32, tag="zc")
        w_all = small_pool.tile([P, 2 * BL], F16, tag="wall")

        # per-item persistent tiles
        x8_sb = x8_pool.tile([P, KD, 16], F8, tag="x8")
        nc.sync.dma_start(x8_sb[:], x8_ap[:])
        xn_rep = []
        for b in range(BL):
            row = xnrow_pool.tile([1, D], F16, tag="xnrow")
            nc.sync.dma_start(row[:], xn_ap[b:b + 1, :])
            rep = xn_pool.tile([P, D], F16, tag=f"xnrep{b}")
            nc.gpsimd.partition_broadcast(rep[:], row[:])
            xn_rep.append(rep)

        G_tiles = {}
        idx32 = idx_pool.tile([P, BL, CJ], I32, tag="idx32")
        nf_all = idx_pool.tile([1, 1], U32, tag="nf")
        mv_cat = mvsb_pool.tile([BL, D], F16, tag="mvcat")

        # ---------------- phase A: fp8 dots on PE ----------------
        def emit_dots(b):
            chunks = {}
            for h in range(2):
                for t in range(KD // 2):
                    ch = mem8_pool.tile([P, 2, N // 2], F8)
                    nc.sync.dma_start(
                        ch[:], memt8_ap[b, :, 2 * t:2 * t + 2,
                                        h * (N // 2):(h + 1) * (N // 2)])
                    chunks[(t, h)] = ch
            for s in range(NSEG):
                h, so = s // 4, (s % 4) * 512
                zps = zps_pool.tile([1, 512], F32)
                for t in range(KD // 2):
                    nc.tensor.matmul(
                        zps[:], x8_sb[:, 2 * t:2 * t + 2, b:b + 1],
                        chunks[(t, h)][:, :, so:so + 512],
                        start=(t == 0), stop=(t == KD // 2 - 1),
                        perf_mode=mybir.MatmulPerfMode.DoubleRow)
                # stage to SBUF, then wrapped copy zw[16b + n//256, n%256]
                zrow = zrow_pool.tile([1, 512], F32, tag="zrow")
                nc.scalar.copy(zrow[:], zps[:])
                for o in range(2):
                    q = WQ * b + 2 * s + o
                    nc.scalar.dma_start(zw[q:q + 1, :],
                                        zrow[0:1, 256 * o:256 * o + 256])

        # ---------------- phase B: coarse newton (one iteration) ----------
        def emit_newton1_iter(g):
            lo, hi = 64 * g, 64 * g + 64
            jr = nsc_pool.tile([P, WF], F32, tag=f"jr{g}")
            nc.scalar.activation(out=jr[lo:hi, :], in_=zw[lo:hi, :], func=AF.Relu,
                                 scale=1.0 / S2, bias=negtau[lo:hi, 0:1],
                                 accum_out=spk[lo:hi, 0:1])
            js = nsc_pool.tile([P, WF], F32, tag=f"js{g}")
            nc.scalar.activation(out=js[lo:hi, :], in_=zw[lo:hi, :], func=AF.Sign,
                                 scale=1.0 / S2, bias=negtau[lo:hi, 0:1],
                                 accum_out=spk[lo:hi, 1:2])
            skpt = sps_pool.tile([P, P], F32, tag="sps")
            skp = skpt[:, 0:2]
            nc.tensor.matmul(skp, g16_sb[:], spk[:], start=True, stop=True)
            nc.scalar.activation(out=kcol[lo:hi], in_=skp[lo:hi, 1:2],
                                 func=AF.Copy, scale=0.5, bias=float(N) / 2.0)
            nc.vector.reciprocal(reck[lo:hi], kcol[lo:hi])
            nc.vector.scalar_tensor_tensor(
                out=dtau[lo:hi], in0=skp[lo:hi, 0:1], scalar=-1.0,
                in1=reck[lo:hi], op0=A.add, op1=A.mult)
            nc.vector.tensor_tensor(out=negtau[lo:hi], in0=negtau[lo:hi],
                                    in1=dtau[lo:hi], op=A.subtract)

        # ---------------- phase C: candidate select/compact/gather --------
        def emit_candidates(g):
            lo, hi = 64 * g, 64 * g + 64
            # thr_scaled = (tau - margin) * S2 = (negtau + margin) * (-S2)
            nc.vector.tensor_scalar(
                out=thrs[lo:hi], in0=negtau[lo:hi], scalar1=MARGIN,
                scalar2=-S2, op0=A.add, op1=A.mult)
            mask = nsc_pool.tile([P, WF], F32, tag=f"mask{g}")
            nc.vector.tensor_scalar(
                out=mask[lo:hi, :], in0=zw[lo:hi, :], scalar1=thrs[lo:hi, 0:1],
                scalar2=None, op0=A.is_gt)
            nc.vector.tensor_tensor(out=v_all[lo:hi, :WF], in0=mask[lo:hi, :],
                                    in1=idxp1[lo:hi, :], op=A.mult)
            nc.vector.tensor_scalar(
                out=v_all[lo:hi, :WF], in0=v_all[lo:hi, :WF], scalar1=1.0,
                scalar2=None, op0=A.subtract)
            for b in range(4 * g, 4 * g + 4):
                qb = WQ * b
                vstage = idx_pool.tile([WQ, WF + SENT_COLS], F32,
                                       tag=f"vstage{b % 2}")
                nc.scalar.dma_start(vstage[:], v_all[qb:qb + WQ, :])
                idxw = idx_pool.tile([WQ, C // WQ], F32, tag=f"idxw{b % 2}")
                nc.gpsimd.sparse_gather(
                    idxw[:], vstage[:], num_found=nf_all[0:1, 0:1])
                idxi = idx_pool.tile([WQ, C // WQ], I32, tag=f"idxi{b % 2}")
                nc.vector.tensor_copy(idxi[:], idxw[:])
                # spread 256 indices one-per-partition: [16,16] -> [128,2]
                for r in range(8):
                    nc.scalar.dma_start(idx32[WQ * r:WQ * r + WQ, b, :],
                                        idxi[:, 2 * r:2 * r + 2])
                G = g_pool.tile([P, CJ, DW], F16, tag=f"G{b}")
                G_tiles[b] = G
                for j in range(CJ):
                    nc.gpsimd.indirect_dma_start(
                        out=G[:, j], out_offset=None,
                        in_=mem16_ap,
                        in_offset=bass.IndirectOffsetOnAxis(
                            ap=idx32[:, b, j:j + 1], axis=0),
                    )

        # ---------------- phase D1: exact per-row dot + rinv extract ------
        def emit_exact_stats(b, j):
            G = G_tiles[b]
            scr = dsc_pool.tile([P, D], F16, tag="dsc")
            nc.vector.scalar_tensor_tensor(
                out=scr[:], in0=G[:, j, 0:D], scalar=1.0, in1=xn_rep[b][:],
                op0=A.mult, op1=A.mult,
                accum_out=dot_all[:, 2 * b + j:2 * b + j + 1])
            nc.vector.tensor_copy(rinv_all[:, 2 * b + j:2 * b + j + 1],
                                  G[:, j, D:D + 1])

        # ---------------- phase D2: exact sparsemax (all items) -----------
        def emit_exact_sparsemax():
            nc.vector.tensor_tensor(out=zc_all[:], in0=dot_all[:],
                                    in1=rinv_all[:], op=A.mult)
            trpt = sps_pool.tile([P, P], F32, tag="sps")
            nc.tensor.transpose(trpt[0:2 * BL, :], zc_all[:], ident32_sb[:])
            nc.vector.tensor_copy(zcT[:], trpt[0:2 * BL, :])
            for _ in range(P2_ITERS):
                jr = nsc_pool.tile([2 * BL, P], F32, tag="jr2")
                nc.scalar.activation(out=jr[:], in_=zcT[:], func=AF.Relu,
                                     bias=negtau2[:, 0:1],
                                     accum_out=spk2[:, 0:1])
                js = nsc_pool.tile([2 * BL, P], F32, tag="js2")
                nc.scalar.activation(out=js[:], in_=zcT[:], func=AF.Sign,
                                     bias=negtau2[:, 0:1],
                                     accum_out=spk2[:, 1:2])
                skpt = sps_pool.tile([P, P], F32, tag="sps")
                skp = skpt[0:2 * BL, 0:2]
                nc.tensor.matmul(skp, g2_sb[:], spk2[:], start=True, stop=True)
                nc.scalar.activation(out=kcol2[:], in_=skp[:, 1:2],
                                     func=AF.Copy, scale=0.5, bias=float(C) / 2.0)
                nc.vector.reciprocal(reck2[:], kcol2[:])
                nc.vector.scalar_tensor_tensor(
                    out=dtau2[:], in0=skp[:, 0:1], scalar=-1.0,
                    in1=reck2[:], op0=A.add, op1=A.mult)
                nc.vector.tensor_tensor(out=negtau2[:], in0=negtau2[:],
                                        in1=dtau2[:], op=A.subtract)
            # replicate -tau_b to all partitions: ntrep[:, c] = negtau2[c]
            nc.vector.tensor_scalar(out=diag16[:], in0=i16_sb[:],
                                    scalar1=negtau2[:, 0:1], scalar2=None,
                                    op0=A.mult)
            ntpst = sps_pool.tile([P, P], F32, tag="sps")
            nc.tensor.matmul(ntpst[:, 0:2 * BL], ones16_128[:], diag16[:],
                             start=True, stop=True)
            nc.vector.tensor_copy(ntrep[:], ntpst[:, 0:2 * BL])
            # weights: w = relu(zc - tau)  (fp16)
            for b in range(BL):
                nc.scalar.activation(out=w_all[:, 2 * b:2 * b + 2],
                                     in_=zc_all[:, 2 * b:2 * b + 2],
                                     func=AF.Relu, bias=ntrep[:, 2 * b:2 * b + 1])

        # ---------------- phase E: weighted sum + h_inT ------------------
        def emit_wsum(b):
            G = G_tiles[b]
            mv_ps = bigps_pool.tile([BL, D], F32, tag="mvps")
            for (s0, sw) in _segments(D, 512):
                for j in range(CJ):
                    nc.tensor.matmul(
                        mv_ps[0:1, s0:s0 + sw], w_all[:, 2 * b + j:2 * b + j + 1],
                        G[:, j, s0:s0 + sw], start=(j == 0), stop=(j == CJ - 1))
            mvrow = mvrow_pool.tile([1, D], F16, tag="mvrow")
            nc.scalar.copy(mvrow[:], mv_ps[0:1, :])
            nc.scalar.dma_start(mv_cat[b:b + 1, :], mvrow[:])

        def emit_hT_mv():
            for kt in range(KD):
                trp = trmv_pool.tile([P, BL], F16, tag="trmv")
                nc.tensor.transpose(trp[:], mv_cat[:, kt * P:(kt + 1) * P],
                                    ident_sb[0:BL, 0:BL])
                nc.vector.tensor_copy(h_inT_mv[:, kt, :], trp[:])

        # ---------------- emission schedule ----------------
        for b in range(4):
            emit_dots(b)
        g0_done = 0
        for b in range(4, BL):
            emit_dots(b)
            if b < 6:
                for _ in range(3):
                    if g0_done < P1_ITERS:
                        emit_newton1_iter(0)
                        g0_done += 1
            elif b == 6:
                while g0_done < P1_ITERS:
                    emit_newton1_iter(0)
                    g0_done += 1
                emit_candidates(0)

        # W1 enc-half streams + early enc-half of matmul1 (into h_enc)
        h_enc = mlp_pool.tile([BL, DHID], F16)
        w1_tiles = {}
        for q in range(QW):
            for k in range(KD):
                wt = w1_pool.tile([P, QWID], F16, tag="w1t")
                nc.sync.dma_start(
                    wt[:], w1t_ap[k * P:(k + 1) * P, q * QWID:(q + 1) * QWID])
                w1_tiles[(q, k)] = wt
        for q in range(QW):
            ps1 = mm1ps_pool.tile([BL, QWID], F32, tag="ps1")
            for k in range(KD):
                wt = w1_tiles[(q, k)]
                for (hs, hw) in _segments(QWID, 512):
                    nc.tensor.matmul(ps1[:, hs:hs + hw], h_inT_enc[:, k, :],
                                     wt[:, hs:hs + hw],
                                     start=(k == 0), stop=False)
            for (hs, hw) in _segments(QWID, 512):
                h0 = q * QWID + hs
                nc.tensor.matmul(ps1[:, hs:hs + hw], ones_row[:],
                                 b1_sb[:, h0:h0 + hw], start=False, stop=True)
                nc.scalar.activation(out=h_enc[:, h0:h0 + hw],
                                     in_=ps1[:, hs:hs + hw], func=AF.Copy)

        # group-1 newton, interleaved with group-0 exact dots
        stats_q = [(bb, j) for bb in range(4) for j in range(CJ)]
        si = 0
        for _ in range(P1_ITERS):
            emit_newton1_iter(1)
            for _ in range(2):
                if si < len(stats_q):
                    emit_exact_stats(*stats_q[si])
                    si += 1
        while si < len(stats_q):
            emit_exact_stats(*stats_q[si])
            si += 1
        emit_candidates(1)
        for bb in range(4, BL):
            for j in range(CJ):
                emit_exact_stats(bb, j)
        # W1 mv-half + W2 stream during the retrieval tail
        for q in range(QW):
            for k in range(KD, KT1):
                wt = w1_pool.tile([P, QWID], F16, tag="w1t")
                nc.sync.dma_start(
                    wt[:], w1t_ap[k * P:(k + 1) * P, q * QWID:(q + 1) * QWID])
                w1_tiles[(q, k)] = wt
        w2_tiles = {}
        for kt in range(KT2):
            wt2 = w2_pool.tile([P, DOUT], F16, tag="w2t")
            nc.sync.dma_start(wt2[:], w2t_ap[kt * P:(kt + 1) * P, :])
            w2_tiles[kt] = wt2

        emit_exact_sparsemax()
        for b in range(BL):
            emit_wsum(b)
        emit_hT_mv()

        # ---------------- MLP tail: mv-half of matmul1, then matmul2 ------
        h_sb = mlp_pool.tile([BL, DHID], F16)
        for q in range(QW):
            ps1 = mm1ps_pool.tile([BL, QWID], F32, tag="ps1")
            for k in range(KD, KT1):
                wt = w1_tiles[(q, k)]
                for (hs, hw) in _segments(QWID, 512):
                    nc.tensor.matmul(ps1[:, hs:hs + hw], h_inT_mv[:, k - KD, :],
                                     wt[:, hs:hs + hw],
                                     start=(k == KD), stop=(k == KT1 - 1))
            for (hs, hw) in _segments(QWID, 512):
                h0 = q * QWID + hs
                hadd = dsc_pool.tile([BL, 512], F16, tag="hadd")
                nc.vector.tensor_tensor(out=hadd[:, 0:hw], in0=ps1[:, hs:hs + hw],
                                        in1=h_enc[:, h0:h0 + hw], op=A.add)
                nc.scalar.activation(out=h_sb[:, h0:h0 + hw], in_=hadd[:, 0:hw],
                                     func=AF.Relu)

        hT_sb = mlp_pool.tile([P, KT2, BL], F16)
        for kt in range(KT2):
            trp = trmv_pool.tile([P, BL], F16, tag="trmv")
            nc.tensor.transpose(trp[:], h_sb[:, kt * P:(kt + 1) * P],
                                ident_sb[0:BL, 0:BL])
            nc.vector.tensor_copy(hT_sb[:, kt, :], trp[:])

        out_sb = mlp_pool.tile([BL, DOUT], F32)
        OSEG2 = _segments(DOUT, 512)
        ps2 = bigps_pool.tile([BL, D], F32, tag="mvps")
        for kt in range(KT2):
            wt2 = w2_tiles[kt]
            for (os_, ow) in OSEG2:
                nc.tensor.matmul(ps2[:, os_:os_ + ow], hT_sb[:, kt, :],
                                 wt2[:, os_:os_ + ow],
                                 start=(kt == 0), stop=False)
        for (os_, ow) in OSEG2:
            nc.tensor.matmul(ps2[:, os_:os_ + ow], ones_row[:],
                             b2_sb[:, os_:os_ + ow], start=False,
                             stop=(os_ + ow >= DOUT))
        nc.scalar.copy(out_sb[:], ps2[:, :DOUT])
        nc.sync.dma_start(out_ap[:], out_sb[:])

    nc.compile()
    return nc


_CACHE = {}


def _get_program(cfg_key):
    if cfg_key not in _CACHE:
        _CACHE[cfg_key] = build_program(FULL_CFG)
    return _CACHE[cfg_key]


def host_prep(encoder_output, memory_set, W1, b1, W2, b2, cfg):
    from ml_dtypes import float8_e4m3fn
    n_cores = cfg["n_cores"]; BL = cfg["b_loc"]; S = cfg["scale"]
    enc = np.asarray(encoder_output, dtype=np.float32)
    mem = np.asarray(memory_set, dtype=np.float32)
    B, N, D = mem.shape
    assert B == n_cores * BL
    nrm = np.maximum(np.sqrt((enc ** 2).sum(-1, keepdims=True)), 1e-6)
    xn = enc / nrm
    mnrm = np.maximum(np.sqrt(np.einsum("bnd,bnd->bn", mem, mem)), 1e-6)
    yn = mem / mnrm[:, :, None]

    def q8(x):
        return np.clip(x * S, -240, 240).astype(float8_e4m3fn)

    # memt8[b, p, s, n] = q8(yn[b, n, 128 s + p])
    yn8 = q8(yn).reshape(B, N, D // P, P)             # [b, n, s, p]
    memt8 = np.ascontiguousarray(yn8.transpose(0, 3, 2, 1))
    x8q = q8(xn).reshape(B, D // P, P)                # [b, s, p]
    x8 = np.zeros((n_cores, P, D // P, 16), dtype=float8_e4m3fn)
    for c in range(n_cores):
        x8[c, :, :, :BL] = x8q[c * BL:(c + 1) * BL].transpose(2, 1, 0)
    # raw fp16 rows + rinv at col 1024; shared zero row at the end
    mem16 = np.zeros((n_cores, BL * N + 1, DW), np.float16)
    mem16[:, :BL * N, :D] = mem.astype(np.float16).reshape(n_cores, BL * N, D)
    mem16[:, :BL * N, D] = (1.0 / mnrm).astype(np.float16).reshape(n_cores, BL * N)
    xn16 = xn.astype(np.float16)
    enct = enc.T.astype(np.float16)
    w1t = np.asarray(W1).T.astype(np.float16)
    w2t = np.asarray(W2).T.astype(np.float16)
    b1r = np.asarray(b1).reshape(1, -1).astype(np.float16)
    b2r = np.asarray(b2).reshape(1, -1).astype(np.float16)
    ident = np.eye(P, dtype=np.float16)
    ident32 = np.eye(P, dtype=np.float32)
    g16 = np.kron(np.eye(P // 16, dtype=np.float32),
                  np.ones((16, 16), np.float32))
    g2 = np.kron(np.eye(BL, dtype=np.float32), np.ones((2, 2), np.float32))
    i16 = np.eye(2 * BL, dtype=np.float32)

    in_maps = []
    for c in range(n_cores):
        sl = slice(c * BL, (c + 1) * BL)
        in_maps.append({
            "memt8": memt8[sl],
            "mem16": mem16[c],
            "x8": x8[c],
            "xn": np.ascontiguousarray(xn16[sl]),
            "enct": np.ascontiguousarray(enct[:, sl]),
            "w1t": w1t, "b1r": b1r, "w2t": w2t, "b2r": b2r,
            "ident": ident, "ident32": ident32,
            "g16": g16, "g2": g2, "i16": i16,
        })
    return in_maps


def kernel(encoder_output, memory_set, W1, b1, W2, b2):
    from concourse.bass_utils import run_bass_kernel_spmd
    cfg = FULL_CFG
    nc = _get_program("full")
    in_maps = host_prep(encoder_output, memory_set, W1, b1, W2, b2, cfg)
    res = run_bass_kernel_spmd(nc, in_maps, core_ids=list(range(cfg["n_cores"])))
    out = np.concatenate([res.results[c]["out"] for c in range(cfg["n_cores"])], axis=0)
    return out.astype(np.float32)
